# revision 28
# baseline (speedup 1.0000x reference)
"""Talking-heads attention kernel for Trainium2, 8-core batch-parallel.

Problem (per batch element b, one NeuronCore each):
  q = x @ Wq; k,v = split(x @ Wkv)
  dots[h,i,j] = (q_h . k_h) * DH**-0.5
  dots' = einsum('hij,hg->gij', dots, mix_pre)
  attn  = softmax(dots', axis=j)
  attn' = einsum('hij,hg->gij', attn, mix_post)
  o_g   = attn'_g @ v_g ;  out = concat(o) @ Wo + bo

Device strategy (per core):
  - ALL inputs packed into ONE DRAM tensor ("blob", [128, BLOB_COLS] bf16)
    laid out exactly as the SBUF destination tile, loaded with a single
    DMA. Per-execute runtime overhead on this stack scales with the
    number of input operands and DMA instructions, so minimizing both
    dominates the wall clock; x is pre-transposed on the host so the
    kernel needs no x-transpose stage.
  - All matmul operands bf16 (PE runs 4x faster than fp32), fp32 PSUM accum.
  - Projections from xT views give qT[d,i], kT[d,j] (zero-padded head
    pairs) and natural v[j,d].
  - QK runs transposed: scoresT[j, (h,i)] per j-tile (j on partitions).
  - Head mixes run on the PE via a transpose sandwich: a PE transpose of
    [j128, (12h x 8i)] chunks puts (h,i8) on partitions; mix_pre is a
    matmul with a host-built block-diagonal weight W1[(h,i1),(g,i2)] =
    mix_pre[h,g]*delta(i1,i2).
  - exp on ACT engine with accum_out giving row sums for free; softmax
    normalization folds into mix_post's stationary weights (W2 scaled
    per-partition by reciprocal sums on DVE).
  - mix_post uses swapped operands (lhsT = exp'd scores chunk, rhs = W2n)
    so its output lands back in [j, (g,i)] layout - undoing the transpose
    for free; AV and the output projection then run natively.
"""

import os

import numpy as np
import ml_dtypes

import concourse.bass as bass
import concourse.tile as tile
from concourse import bacc
from concourse import mybir
from concourse.masks import make_identity

B, N, DIM, H, DH = 8, 1024, 768, 12, 64
INNER = H * DH  # 768
NCC = DIM // 128  # 6 chunks of the contraction/feature dims
NJT = N // 128  # 8 j-tiles
IB = 128  # i-block (query rows per block)
NBLK = N // IB  # 8
ISUB = 8  # i positions packed with the 12 heads: K = 12*8 = 96
NGRP = IB // ISUB  # 16 i8-groups per block
KP = H * ISUB  # 96 partitions used in mix land

BF16 = mybir.dt.bfloat16
F32 = mybir.dt.float32

# blob column layout (all bf16, [128, BLOB_COLS])
C_XT = 0                      # 6 chunks [128,1024] of x^T
C_WQ = C_XT + NCC * N         # 6 chunks [128,768] of Wq*scale
C_WK = C_WQ + NCC * INNER
C_WV = C_WK + NCC * INNER
C_WO = C_WV + NCC * INNER
C_W1 = C_WO + NCC * DIM       # [96,96] on partitions 0..95
C_W2 = C_W1 + KP              # [96,96] on partitions 0..95
C_BO = C_W2 + KP              # [128,768] broadcast bias
BLOB_COLS = C_BO + DIM

_cache = {}
KSTAGE = int(os.environ.get("KSTAGE", "9"))
KALT = int(os.environ.get("KALT", "1"))
KBLKS = int(os.environ.get("KBLKS", str(NBLK)))


def _build():
    nc = bacc.Bacc("TRN2", target_bir_lowering=False, debug=False,
                   enable_partition_id=False)
    d_blob = nc.dram_tensor("blob", [128, BLOB_COLS], BF16,
                            kind="ExternalInput").ap()
    d_out = nc.dram_tensor("out", [N, DIM], BF16, kind="ExternalOutput").ap()
    with tile.TileContext(nc) as tc:
        _body(tc, d_out, d_blob)
    nc.compile()
    return nc


def _body(tc, d_out, d_blob):
    nc = tc.nc
    from contextlib import ExitStack

    ctx = ExitStack()
    with ctx:
        consts = ctx.enter_context(tc.tile_pool(name="consts", bufs=1))
        sbig = ctx.enter_context(tc.tile_pool(name="sbig", bufs=1))
        stream = ctx.enter_context(tc.tile_pool(name="stream", bufs=4))
        small = ctx.enter_context(tc.tile_pool(name="small", bufs=6))
        # PSUM: 8 banks total, each slot = 1 bank. 2+2+2+2 = 8.
        ps_misc = ctx.enter_context(tc.tile_pool(name="ps_misc", bufs=3, space="PSUM"))
        ps_t1 = ctx.enter_context(tc.tile_pool(name="ps_t1", bufs=2, space="PSUM"))
        ps_m1 = ctx.enter_context(tc.tile_pool(name="ps_m1", bufs=1, space="PSUM"))
        ps_m2 = ctx.enter_context(tc.tile_pool(name="ps_m2", bufs=1, space="PSUM"))

        def psum_mm():
            return ps_misc.tile([128, 512], F32, name="mm", tag="mm")

        # ---- constants ----
        ident = consts.tile([128, 128], BF16, name="ident", tag="ident")
        make_identity(nc, ident)

        blob = consts.tile([128, BLOB_COLS], BF16, name="blob", tag="blob")
        nc.sync.dma_start(blob[:], d_blob[:, :])
        xT = [blob[:, C_XT + i * N: C_XT + (i + 1) * N] for i in range(NCC)]
        wq = [blob[:, C_WQ + i * INNER: C_WQ + (i + 1) * INNER] for i in range(NCC)]
        wk = [blob[:, C_WK + i * INNER: C_WK + (i + 1) * INNER] for i in range(NCC)]
        wv = [blob[:, C_WV + i * INNER: C_WV + (i + 1) * INNER] for i in range(NCC)]
        wo = [blob[:, C_WO + i * DIM: C_WO + (i + 1) * DIM] for i in range(NCC)]
        w1 = blob[0:KP, C_W1: C_W1 + KP]
        w2 = blob[0:KP, C_W2: C_W2 + KP]
        bo_b = blob[:, C_BO: C_BO + DIM]

        # ---- projections ----
        qT = [sbig.tile([128, N], BF16, name=f"qT{i}", tag=f"qT{i}") for i in range(NCC)]
        kTz = [[sbig.tile([128, N], BF16, name=f"kTz{p}{i}", tag=f"kTz{p}{i}")
                for i in range(NCC)] for p in range(2)]
        for i in range(NCC):
            nc.vector.memset(kTz[0][i][64:128, :], 0.0)
            nc.vector.memset(kTz[1][i][0:64, :], 0.0)
        v = [sbig.tile([128, INNER], BF16, name=f"v{i}", tag=f"v{i}") for i in range(NJT)]
        for which, w in (("q", wq), ("k", wk)):
            for dc in range(NCC):
                for ih in range(2):
                    pp = psum_mm()
                    for cc in range(NCC):
                        nc.tensor.matmul(
                            pp[:],
                            lhsT=w[cc][:, dc * 128:(dc + 1) * 128],
                            rhs=xT[cc][:, ih * 512:(ih + 1) * 512],
                            start=(cc == 0), stop=(cc == NCC - 1),
                        )
                    sl = slice(ih * 512, (ih + 1) * 512)
                    if which == "q":
                        if dc % 2 == 0:
                            nc.vector.tensor_copy(qT[dc][:, sl], pp[:])
                        else:
                            nc.scalar.copy(qT[dc][:, sl], pp[:])
                    else:
                        nc.vector.tensor_copy(kTz[0][dc][0:64, sl], pp[0:64, :])
                        nc.scalar.copy(kTz[1][dc][64:128, sl], pp[64:128, :])
        for jt in range(NJT):
            for d0, dn in ((0, 512), (512, 256)):
                pp = psum_mm()
                for cc in range(NCC):
                    nc.tensor.matmul(
                        pp[:, :dn],
                        lhsT=xT[cc][:, jt * 128:(jt + 1) * 128],
                        rhs=wv[cc][:, d0:d0 + dn],
                        start=(cc == 0), stop=(cc == NCC - 1),
                    )
                if jt % 2 == 0:
                    nc.vector.tensor_copy(v[jt][:, d0:d0 + dn], pp[:, :dn])
                else:
                    nc.scalar.copy(v[jt][:, d0:d0 + dn], pp[:, :dn])

        if KSTAGE <= 1:
            dbg = stream.tile([128, DIM], BF16, name="dbg", tag="out_sb")
            for it in range(8):
                nc.vector.tensor_copy(dbg[:], qT[it % NCC][:, :DIM])
                nc.sync.dma_start(d_out[it * 128:(it + 1) * 128, :], dbg[:])
            return

        # ---- main attention blocks ----
        s_all = [sbig.tile([128, H * IB], BF16, name=f"s{jt}", tag=f"s{jt}") for jt in range(NJT)]
        a_all = sbig.tile([128, NJT * H * IB], BF16, name="a_all", tag="a_all")
        oT = [sbig.tile([128, N], BF16, name=f"oT{i}", tag=f"oT{i}") for i in range(NCC)]
        outp = ctx.enter_context(tc.tile_pool(name="outp", bufs=2))

        for bi in range(KBLKS):
            # -- QK: scoresT[j, (h,i)] --
            for jt in range(NJT):
                for hs in range(3):  # 4 heads per psum bank
                    pq = psum_mm()
                    for hh in range(4):
                        h = hs * 4 + hh
                        dc, p = h // 2, h % 2
                        nc.tensor.matmul(
                            pq[:, hh * IB:(hh + 1) * IB],
                            lhsT=kTz[p][dc][:, jt * 128:(jt + 1) * 128],
                            rhs=qT[dc][:, bi * IB:(bi + 1) * IB],
                            start=True, stop=True,
                        )
                    sdst = s_all[jt][:].rearrange(
                        "p (g h i) -> p g h i", g=NGRP, h=H)[
                        :, :, hs * 4:(hs + 1) * 4, :]
                    ssrc = pq[:].rearrange("p (h g i) -> p g h i", h=4, g=NGRP)
                    if KALT == 0 or (jt + hs) % 2 == 0:
                        nc.vector.tensor_copy(sdst, ssrc)
                    else:
                        nc.scalar.copy(sdst, ssrc)

            if KSTAGE <= 2:
                dbg2 = stream.tile([128, DIM], BF16, name="dbg2", tag="out_sb")
                nc.vector.tensor_copy(dbg2[:], s_all[bi][:, :DIM])
                nc.sync.dma_start(d_out[bi * IB:(bi + 1) * IB, :], dbg2[:])
                continue

            # -- per i8-group: transpose -> mix1 -> exp -> mix2(swap) --
            for g in range(NGRP):
                ts_sb = stream.tile([KP, N], BF16, name="ts", tag="ts")
                for jh in range(2):
                    pt = ps_t1.tile([KP, 512], BF16, name="ps_t1", tag="ps_t1")
                    for jr in range(4):
                        jt = jh * 4 + jr
                        nc.tensor.transpose(
                            pt[:, jr * 128:(jr + 1) * 128],
                            s_all[jt][:, g * KP:(g + 1) * KP], ident[:])
                    nc.vector.tensor_copy(
                        ts_sb[:, jh * 512:(jh + 1) * 512], pt[:])

                e_sb = stream.tile([KP, N], BF16, name="e", tag="e")
                ssum = small.tile([KP, 1], F32, name="ssum", tag="ssum")
                pm = ps_m1.tile([KP, 1024], F32, name="ps_m1", tag="ps_m1")
                for jh in range(2):
                    nc.tensor.matmul(
                        pm[:, jh * 512:(jh + 1) * 512],
                        lhsT=w1, rhs=ts_sb[:, jh * 512:(jh + 1) * 512],
                        start=True, stop=True,
                    )
                nc.scalar.activation(
                    e_sb[:], pm[:],
                    mybir.ActivationFunctionType.Exp,
                    accum_out=ssum[:],
                )
                rsum = small.tile([KP, 1], F32, name="rsum", tag="rsum")
                nc.vector.reciprocal(rsum[:], ssum[:])
                w2n = small.tile([KP, KP], BF16, name="w2n", tag="w2n")
                nc.vector.tensor_scalar_mul(w2n[:], w2, rsum[:])

                # mix2 swapped: out[j, (g', i2)]; psum packs 4 j-tiles per bank
                for jh in range(2):
                    pm2 = ps_m2.tile([128, 4 * KP], F32, name="ps_m2", tag="ps_m2")
                    for jr in range(4):
                        nc.tensor.matmul(
                            pm2[:, jr * KP:(jr + 1) * KP],
                            lhsT=e_sb[:, (jh * 4 + jr) * 128:(jh * 4 + jr + 1) * 128],
                            rhs=w2n[:],
                            start=True, stop=True,
                        )
                    # dest: a_all[p, jt*H*IB + gp*IB + (g*ISUB + i2)]
                    a_r = a_all[:].rearrange("p (t h i) -> p t h i", t=NJT, h=H)
                    dst = a_r[:, jh * 4:(jh + 1) * 4, :, g * ISUB:(g + 1) * ISUB]
                    src = pm2[:].rearrange("p (t h i) -> p t h i", t=4, h=H)
                    if jh == 1 and g % 2 == 1:
                        nc.scalar.copy(dst, src)
                    else:
                        nc.vector.tensor_copy(dst, src)

            if KSTAGE <= 3:
                dbg3 = stream.tile([128, DIM], BF16, name="dbg3", tag="out_sb")
                nc.vector.tensor_copy(
                    dbg3[:], a_all[:].rearrange("p (t x) -> p t x", t=NJT)[:, 0, :DIM])
                nc.sync.dma_start(d_out[bi * IB:(bi + 1) * IB, :], dbg3[:])
                continue

            # -- AV: oT[(g',d), i] --
            for gp in range(H):
                pav = ps_misc.tile([64, 128], F32, name="mm", tag="mm")
                a_r = a_all[:].rearrange("p (t h i) -> p t h i", t=NJT, h=H)
                for jt in range(NJT):
                    rhs = a_r[:, jt, gp, :]
                    nc.tensor.matmul(
                        pav[:],
                        lhsT=v[jt][:, gp * 64:(gp + 1) * 64],
                        rhs=rhs,
                        start=(jt == 0), stop=(jt == NJT - 1),
                    )
                dc, dp = gp // 2, (gp % 2) * 64
                nc.vector.tensor_copy(
                    oT[dc][dp:dp + 64, bi * IB:(bi + 1) * IB], pav[:])

            # -- output projection + bias; 4 blocks batched per store DMA --
            if bi % 4 == 0:
                out_sb4 = outp.tile([128, 4 * DIM], BF16, name="out_sb4",
                                    tag="out_sb4")
            for e0, en in ((0, 512), (512, 256)):
                po = psum_mm()
                for gc in range(NCC):
                    nc.tensor.matmul(
                        po[:, :en],
                        lhsT=oT[gc][:, bi * IB:(bi + 1) * IB],
                        rhs=wo[gc][:, e0:e0 + en],
                        start=(gc == 0), stop=(gc == NCC - 1),
                    )
                nc.vector.tensor_add(
                    out_sb4[:, (bi % 4) * DIM + e0:(bi % 4) * DIM + e0 + en],
                    po[:, :en], bo_b[:, e0:e0 + en])
            if bi % 4 == 3:
                src = out_sb4[:].rearrange("p (t e) -> p t e", t=4)
                dst = d_out[(bi - 3) * IB:(bi + 1) * IB, :].rearrange(
                    "(t p) e -> p t e", t=4)
                nc.sync.dma_start(dst, src)


def _prep_weights(Wq, Wkv, mix_pre, mix_post, Wo, bo):
    """Weight section of the blob (identical for every core), as bf16."""
    bf16 = ml_dtypes.bfloat16
    scale = DH ** -0.5
    wq = (np.asarray(Wq, np.float32) * scale).astype(bf16)
    wkv = np.asarray(Wkv, np.float32)
    wk = wkv[:, :INNER].astype(bf16)
    wv = wkv[:, INNER:].astype(bf16)
    wo = np.asarray(Wo, np.float32).astype(bf16)
    m1 = np.asarray(mix_pre, np.float32)
    m2 = np.asarray(mix_post, np.float32)
    eye = np.eye(ISUB, dtype=np.float32)
    # W1[(h,i1),(g,i2)] = mix_pre[h,g] * delta(i1,i2)
    w1 = np.einsum("hg,ab->hagb", m1, eye).reshape(KP, KP).astype(bf16)
    w2 = np.einsum("hg,ab->hagb", m2, eye).reshape(KP, KP).astype(bf16)
    bo_ = np.asarray(bo, np.float32).astype(bf16)

    wsec = np.zeros((128, BLOB_COLS - C_WQ), bf16)
    off = 0

    def chunks(m, width):  # [768, width] -> [128, 6*width]
        return m.reshape(NCC, 128, width).transpose(1, 0, 2).reshape(128, NCC * width)

    wsec[:, off:off + NCC * INNER] = chunks(wq, INNER); off += NCC * INNER
    wsec[:, off:off + NCC * INNER] = chunks(wk, INNER); off += NCC * INNER
    wsec[:, off:off + NCC * INNER] = chunks(wv, INNER); off += NCC * INNER
    wsec[:, off:off + NCC * DIM] = chunks(wo, DIM); off += NCC * DIM
    wsec[0:KP, off:off + KP] = w1; off += KP
    wsec[0:KP, off:off + KP] = w2; off += KP
    wsec[:, off:off + DIM] = np.broadcast_to(bo_, (128, DIM)); off += DIM
    assert off == BLOB_COLS - C_WQ
    return wsec


def _prep_inputs(x, Wq, Wkv, mix_pre, mix_post, Wo, bo):
    bf16 = ml_dtypes.bfloat16
    wsec = _prep_weights(Wq, Wkv, mix_pre, mix_post, Wo, bo)
    x = np.asarray(x, np.float32)
    in_maps = []
    for b in range(B):
        blob = np.empty((128, BLOB_COLS), bf16)
        xt = x[b].T.astype(bf16)  # [768, 1024]
        blob[:, :C_WQ] = xt.reshape(NCC, 128, N).transpose(1, 0, 2).reshape(128, NCC * N)
        blob[:, C_WQ:] = wsec
        in_maps.append({"blob": blob})
    return in_maps


def _get_nc():
    if "nc" not in _cache:
        _cache["nc"] = _build()
    return _cache["nc"]


def _get_runner():
    """Persistent jitted 8-core runner (jit built once, reused every call)."""
    if "runner" in _cache:
        return _cache["runner"]
    import jax
    from concourse import bass2jax
    from jax.sharding import Mesh, PartitionSpec
    from jax.experimental.shard_map import shard_map

    nc = _get_nc()
    bass2jax.install_neuronx_cc_hook()
    out_aval = jax.core.ShapedArray((N, DIM), ml_dtypes.bfloat16)

    def _bass_body(blob_arg, out_zero):
        outs = bass2jax._bass_exec_p.bind(
            blob_arg, out_zero,
            out_avals=(out_aval,),
            in_names=("blob", "out"),
            out_names=("out",),
            lowering_input_output_aliases=(),
            sim_require_finite=True,
            sim_require_nnan=True,
            nc=nc,
        )
        return tuple(outs)

    devices = jax.devices()[:B]
    mesh = Mesh(np.asarray(devices), ("core",))
    fn = jax.jit(shard_map(_bass_body, mesh=mesh,
                           in_specs=(PartitionSpec("core"),) * 2,
                           out_specs=(PartitionSpec("core"),),
                           check_rep=False),
                 keep_unused=True)
    zeros = np.zeros((B * N, DIM), ml_dtypes.bfloat16)

    def run(blobs):  # blobs: [B*128, BLOB_COLS] bf16
        out = fn(blobs, zeros)[0]
        return np.asarray(out).reshape(B, N, DIM)

    _cache["runner"] = run
    return run


def kernel(x, Wq, Wkv, mix_pre, mix_post, Wo, bo):
    run = _get_runner()
    in_maps = _prep_inputs(x, Wq, Wkv, mix_pre, mix_post, Wo, bo)
    blobs = np.concatenate([m["blob"] for m in in_maps], axis=0)
    return run(blobs).astype(np.float32)


# revision 29
# speedup vs baseline: 1.1294x; 1.1294x over previous
"""Talking-heads attention kernel for Trainium2, 8-core batch-parallel.

Problem (per batch element b, one NeuronCore each):
  q = x @ Wq; k,v = split(x @ Wkv)
  dots[h,i,j] = (q_h . k_h) * DH**-0.5
  dots' = einsum('hij,hg->gij', dots, mix_pre)
  attn  = softmax(dots', axis=j)
  attn' = einsum('hij,hg->gij', attn, mix_post)
  o_g   = attn'_g @ v_g ;  out = concat(o) @ Wo + bo

Device strategy (per core):
  - ALL inputs packed into ONE DRAM tensor ("blob", [128, BLOB_COLS] bf16)
    laid out exactly as the SBUF destination tile, loaded with a single
    DMA. Per-execute runtime overhead on this stack scales with the
    number of input operands and DMA instructions, so minimizing both
    dominates the wall clock; x is pre-transposed on the host so the
    kernel needs no x-transpose stage.
  - All matmul operands bf16 (PE runs 4x faster than fp32), fp32 PSUM accum.
  - Projections from xT views give qT[d,i], kT[d,j] (zero-padded head
    pairs) and natural v[j,d].
  - QK runs transposed: scoresT[j, (h,i)] per j-tile (j on partitions).
  - Head mixes run on the PE via a transpose sandwich: a PE transpose of
    [j128, (12h x 8i)] chunks puts (h,i8) on partitions; mix_pre is a
    matmul with a host-built block-diagonal weight W1[(h,i1),(g,i2)] =
    mix_pre[h,g]*delta(i1,i2).
  - exp on ACT engine with accum_out giving row sums for free; softmax
    normalization folds into mix_post's stationary weights (W2 scaled
    per-partition by reciprocal sums on DVE).
  - mix_post uses swapped operands (lhsT = exp'd scores chunk, rhs = W2n)
    so its output lands back in [j, (g,i)] layout - undoing the transpose
    for free; AV and the output projection then run natively.
"""

import os

import numpy as np
import ml_dtypes

import concourse.bass as bass
import concourse.tile as tile
from concourse import bacc
from concourse import mybir
from concourse.masks import make_identity

B, N, DIM, H, DH = 8, 1024, 768, 12, 64
INNER = H * DH  # 768
NCC = DIM // 128  # 6 chunks of the contraction/feature dims
NJT = N // 128  # 8 j-tiles
IB = 128  # i-block (query rows per block)
NBLK = N // IB  # 8
ISUB = 8  # i positions packed with the 12 heads: K = 12*8 = 96
NGRP = IB // ISUB  # 16 i8-groups per block
KP = H * ISUB  # 96 partitions used in mix land

BF16 = mybir.dt.bfloat16
F32 = mybir.dt.float32

# Weight-section column layout (bf16, [128, WSEC_COLS]); identical on all
# cores, so each core ships 1/8th and an on-device AllGather reassembles it
# (per-execute runtime overhead scales with shipped input bytes).
W_WQ = 0                      # 6 chunks [128,768] of Wq*scale
W_WK = W_WQ + NCC * INNER
W_WV = W_WK + NCC * INNER
W_WO = W_WV + NCC * INNER
W_W1 = W_WO + NCC * DIM       # [96,96] on partitions 0..95
W_W2 = W_W1 + KP              # [96,96] on partitions 0..95
W_BO = W_W2 + KP              # [128,768] broadcast bias
WSEC_COLS = W_BO + DIM        # 19392, divisible by 8
WSH_COLS = WSEC_COLS // B     # 2424 per-core weight shard

# blob column layout (all bf16, [128, BLOB_COLS])
C_XT = 0                      # 6 chunks [128,1024] of x^T
C_WSH = C_XT + NCC * N        # this core's weight shard
BLOB_COLS = C_WSH + WSH_COLS

_cache = {}
KSTAGE = int(os.environ.get("KSTAGE", "9"))
KALT = int(os.environ.get("KALT", "1"))
KBLKS = int(os.environ.get("KBLKS", str(NBLK)))


def _build():
    nc = bacc.Bacc("TRN2", target_bir_lowering=False, debug=False,
                   enable_partition_id=False, num_devices=B)
    d_blob = nc.dram_tensor("blob", [128, BLOB_COLS], BF16,
                            kind="ExternalInput").ap()
    d_out = nc.dram_tensor("out", [N, DIM], BF16, kind="ExternalOutput").ap()
    with tile.TileContext(nc) as tc:
        _body(tc, d_out, d_blob)
    nc.compile()
    return nc


def _body(tc, d_out, d_blob):
    nc = tc.nc
    from contextlib import ExitStack

    ctx = ExitStack()
    with ctx:
        consts = ctx.enter_context(tc.tile_pool(name="consts", bufs=1))
        sbig = ctx.enter_context(tc.tile_pool(name="sbig", bufs=1))
        stream = ctx.enter_context(tc.tile_pool(name="stream", bufs=4))
        small = ctx.enter_context(tc.tile_pool(name="small", bufs=6))
        # PSUM: 8 banks total, each slot = 1 bank. 2+2+2+2 = 8.
        ps_misc = ctx.enter_context(tc.tile_pool(name="ps_misc", bufs=3, space="PSUM"))
        ps_t1 = ctx.enter_context(tc.tile_pool(name="ps_t1", bufs=2, space="PSUM"))
        ps_m1 = ctx.enter_context(tc.tile_pool(name="ps_m1", bufs=1, space="PSUM"))
        ps_m2 = ctx.enter_context(tc.tile_pool(name="ps_m2", bufs=1, space="PSUM"))

        def psum_mm():
            return ps_misc.tile([128, 512], F32, name="mm", tag="mm")

        # ---- constants ----
        ident = consts.tile([128, 128], BF16, name="ident", tag="ident")
        make_identity(nc, ident)

        blob = consts.tile([128, BLOB_COLS], BF16, name="blob", tag="blob")
        nc.sync.dma_start(blob[:], d_blob[:, :])
        xT = [blob[:, C_XT + i * N: C_XT + (i + 1) * N] for i in range(NCC)]

        # AllGather the weight shards: every core contributes [128, WSH_COLS]
        # and reads back the full weight section into SBUF.
        dram = ctx.enter_context(tc.tile_pool(name="dram", bufs=1, space="DRAM"))
        wsh_bounce = dram.tile([128, WSH_COLS], BF16, name="wshb", tag="wshb")
        wg_bounce = dram.tile([B * 128, WSH_COLS], BF16, name="wgb", tag="wgb")
        nc.gpsimd.dma_start(wsh_bounce[:], d_blob[:, C_WSH:C_WSH + WSH_COLS])
        nc.gpsimd.collective_compute(
            "AllGather",
            mybir.AluOpType.bypass,
            replica_groups=[list(range(B))],
            ins=[wsh_bounce.opt()],
            outs=[wg_bounce.opt()],
        )
        wsb = consts.tile([128, WSEC_COLS], BF16, name="wsb", tag="wsb")
        nc.sync.dma_start(
            wsb[:].rearrange("p (r c) -> p r c", r=B),
            wg_bounce[:].rearrange("(r p) c -> p r c", r=B),
        )
        wq = [wsb[:, W_WQ + i * INNER: W_WQ + (i + 1) * INNER] for i in range(NCC)]
        wk = [wsb[:, W_WK + i * INNER: W_WK + (i + 1) * INNER] for i in range(NCC)]
        wv = [wsb[:, W_WV + i * INNER: W_WV + (i + 1) * INNER] for i in range(NCC)]
        wo = [wsb[:, W_WO + i * DIM: W_WO + (i + 1) * DIM] for i in range(NCC)]
        w1 = wsb[0:KP, W_W1: W_W1 + KP]
        w2 = wsb[0:KP, W_W2: W_W2 + KP]
        bo_b = wsb[:, W_BO: W_BO + DIM]

        # ---- projections ----
        qT = [sbig.tile([128, N], BF16, name=f"qT{i}", tag=f"qT{i}") for i in range(NCC)]
        kTz = [[sbig.tile([128, N], BF16, name=f"kTz{p}{i}", tag=f"kTz{p}{i}")
                for i in range(NCC)] for p in range(2)]
        for i in range(NCC):
            nc.vector.memset(kTz[0][i][64:128, :], 0.0)
            nc.vector.memset(kTz[1][i][0:64, :], 0.0)
        v = [sbig.tile([128, INNER], BF16, name=f"v{i}", tag=f"v{i}") for i in range(NJT)]
        for which, w in (("q", wq), ("k", wk)):
            for dc in range(NCC):
                for ih in range(2):
                    pp = psum_mm()
                    for cc in range(NCC):
                        nc.tensor.matmul(
                            pp[:],
                            lhsT=w[cc][:, dc * 128:(dc + 1) * 128],
                            rhs=xT[cc][:, ih * 512:(ih + 1) * 512],
                            start=(cc == 0), stop=(cc == NCC - 1),
                        )
                    sl = slice(ih * 512, (ih + 1) * 512)
                    if which == "q":
                        if dc % 2 == 0:
                            nc.vector.tensor_copy(qT[dc][:, sl], pp[:])
                        else:
                            nc.scalar.copy(qT[dc][:, sl], pp[:])
                    else:
                        nc.vector.tensor_copy(kTz[0][dc][0:64, sl], pp[0:64, :])
                        nc.scalar.copy(kTz[1][dc][64:128, sl], pp[64:128, :])
        for jt in range(NJT):
            for d0, dn in ((0, 512), (512, 256)):
                pp = psum_mm()
                for cc in range(NCC):
                    nc.tensor.matmul(
                        pp[:, :dn],
                        lhsT=xT[cc][:, jt * 128:(jt + 1) * 128],
                        rhs=wv[cc][:, d0:d0 + dn],
                        start=(cc == 0), stop=(cc == NCC - 1),
                    )
                if jt % 2 == 0:
                    nc.vector.tensor_copy(v[jt][:, d0:d0 + dn], pp[:, :dn])
                else:
                    nc.scalar.copy(v[jt][:, d0:d0 + dn], pp[:, :dn])

        if KSTAGE <= 1:
            dbg = stream.tile([128, DIM], BF16, name="dbg", tag="out_sb")
            for it in range(8):
                nc.vector.tensor_copy(dbg[:], qT[it % NCC][:, :DIM])
                nc.sync.dma_start(d_out[it * 128:(it + 1) * 128, :], dbg[:])
            return

        # ---- main attention blocks ----
        s_all = [sbig.tile([128, H * IB], BF16, name=f"s{jt}", tag=f"s{jt}") for jt in range(NJT)]
        a_all = sbig.tile([128, NJT * H * IB], BF16, name="a_all", tag="a_all")
        oT = [sbig.tile([128, N], BF16, name=f"oT{i}", tag=f"oT{i}") for i in range(NCC)]
        outp = ctx.enter_context(tc.tile_pool(name="outp", bufs=2))

        for bi in range(KBLKS):
            # -- QK: scoresT[j, (h,i)] --
            for jt in range(NJT):
                for hs in range(3):  # 4 heads per psum bank
                    pq = psum_mm()
                    for hh in range(4):
                        h = hs * 4 + hh
                        dc, p = h // 2, h % 2
                        nc.tensor.matmul(
                            pq[:, hh * IB:(hh + 1) * IB],
                            lhsT=kTz[p][dc][:, jt * 128:(jt + 1) * 128],
                            rhs=qT[dc][:, bi * IB:(bi + 1) * IB],
                            start=True, stop=True,
                        )
                    sdst = s_all[jt][:].rearrange(
                        "p (g h i) -> p g h i", g=NGRP, h=H)[
                        :, :, hs * 4:(hs + 1) * 4, :]
                    ssrc = pq[:].rearrange("p (h g i) -> p g h i", h=4, g=NGRP)
                    if KALT == 0 or (jt + hs) % 2 == 0:
                        nc.vector.tensor_copy(sdst, ssrc)
                    else:
                        nc.scalar.copy(sdst, ssrc)

            if KSTAGE <= 2:
                dbg2 = stream.tile([128, DIM], BF16, name="dbg2", tag="out_sb")
                nc.vector.tensor_copy(dbg2[:], s_all[bi][:, :DIM])
                nc.sync.dma_start(d_out[bi * IB:(bi + 1) * IB, :], dbg2[:])
                continue

            # -- per i8-group: transpose -> mix1 -> exp -> mix2(swap) --
            for g in range(NGRP):
                ts_sb = stream.tile([KP, N], BF16, name="ts", tag="ts")
                for jh in range(2):
                    pt = ps_t1.tile([KP, 512], BF16, name="ps_t1", tag="ps_t1")
                    for jr in range(4):
                        jt = jh * 4 + jr
                        nc.tensor.transpose(
                            pt[:, jr * 128:(jr + 1) * 128],
                            s_all[jt][:, g * KP:(g + 1) * KP], ident[:])
                    nc.vector.tensor_copy(
                        ts_sb[:, jh * 512:(jh + 1) * 512], pt[:])

                e_sb = stream.tile([KP, N], BF16, name="e", tag="e")
                ssum = small.tile([KP, 1], F32, name="ssum", tag="ssum")
                pm = ps_m1.tile([KP, 1024], F32, name="ps_m1", tag="ps_m1")
                for jh in range(2):
                    nc.tensor.matmul(
                        pm[:, jh * 512:(jh + 1) * 512],
                        lhsT=w1, rhs=ts_sb[:, jh * 512:(jh + 1) * 512],
                        start=True, stop=True,
                    )
                nc.scalar.activation(
                    e_sb[:], pm[:],
                    mybir.ActivationFunctionType.Exp,
                    accum_out=ssum[:],
                )
                rsum = small.tile([KP, 1], F32, name="rsum", tag="rsum")
                nc.vector.reciprocal(rsum[:], ssum[:])
                w2n = small.tile([KP, KP], BF16, name="w2n", tag="w2n")
                nc.vector.tensor_scalar_mul(w2n[:], w2, rsum[:])

                # mix2 swapped: out[j, (g', i2)]; psum packs 4 j-tiles per bank
                for jh in range(2):
                    pm2 = ps_m2.tile([128, 4 * KP], F32, name="ps_m2", tag="ps_m2")
                    for jr in range(4):
                        nc.tensor.matmul(
                            pm2[:, jr * KP:(jr + 1) * KP],
                            lhsT=e_sb[:, (jh * 4 + jr) * 128:(jh * 4 + jr + 1) * 128],
                            rhs=w2n[:],
                            start=True, stop=True,
                        )
                    # dest: a_all[p, jt*H*IB + gp*IB + (g*ISUB + i2)]
                    a_r = a_all[:].rearrange("p (t h i) -> p t h i", t=NJT, h=H)
                    dst = a_r[:, jh * 4:(jh + 1) * 4, :, g * ISUB:(g + 1) * ISUB]
                    src = pm2[:].rearrange("p (t h i) -> p t h i", t=4, h=H)
                    if jh == 1 and g % 2 == 1:
                        nc.scalar.copy(dst, src)
                    else:
                        nc.vector.tensor_copy(dst, src)

            if KSTAGE <= 3:
                dbg3 = stream.tile([128, DIM], BF16, name="dbg3", tag="out_sb")
                nc.vector.tensor_copy(
                    dbg3[:], a_all[:].rearrange("p (t x) -> p t x", t=NJT)[:, 0, :DIM])
                nc.sync.dma_start(d_out[bi * IB:(bi + 1) * IB, :], dbg3[:])
                continue

            # -- AV: oT[(g',d), i] --
            for gp in range(H):
                pav = ps_misc.tile([64, 128], F32, name="mm", tag="mm")
                a_r = a_all[:].rearrange("p (t h i) -> p t h i", t=NJT, h=H)
                for jt in range(NJT):
                    rhs = a_r[:, jt, gp, :]
                    nc.tensor.matmul(
                        pav[:],
                        lhsT=v[jt][:, gp * 64:(gp + 1) * 64],
                        rhs=rhs,
                        start=(jt == 0), stop=(jt == NJT - 1),
                    )
                dc, dp = gp // 2, (gp % 2) * 64
                nc.vector.tensor_copy(
                    oT[dc][dp:dp + 64, bi * IB:(bi + 1) * IB], pav[:])

            # -- output projection + bias; 4 blocks batched per store DMA --
            if bi % 4 == 0:
                out_sb4 = outp.tile([128, 4 * DIM], BF16, name="out_sb4",
                                    tag="out_sb4")
            for e0, en in ((0, 512), (512, 256)):
                po = psum_mm()
                for gc in range(NCC):
                    nc.tensor.matmul(
                        po[:, :en],
                        lhsT=oT[gc][:, bi * IB:(bi + 1) * IB],
                        rhs=wo[gc][:, e0:e0 + en],
                        start=(gc == 0), stop=(gc == NCC - 1),
                    )
                nc.vector.tensor_add(
                    out_sb4[:, (bi % 4) * DIM + e0:(bi % 4) * DIM + e0 + en],
                    po[:, :en], bo_b[:, e0:e0 + en])
            if bi % 4 == 3:
                src = out_sb4[:].rearrange("p (t e) -> p t e", t=4)
                dst = d_out[(bi - 3) * IB:(bi + 1) * IB, :].rearrange(
                    "(t p) e -> p t e", t=4)
                nc.sync.dma_start(dst, src)


def _prep_weights(Wq, Wkv, mix_pre, mix_post, Wo, bo):
    """Weight section of the blob (identical for every core), as bf16."""
    bf16 = ml_dtypes.bfloat16
    scale = DH ** -0.5
    wq = (np.asarray(Wq, np.float32) * scale).astype(bf16)
    wkv = np.asarray(Wkv, np.float32)
    wk = wkv[:, :INNER].astype(bf16)
    wv = wkv[:, INNER:].astype(bf16)
    wo = np.asarray(Wo, np.float32).astype(bf16)
    m1 = np.asarray(mix_pre, np.float32)
    m2 = np.asarray(mix_post, np.float32)
    eye = np.eye(ISUB, dtype=np.float32)
    # W1[(h,i1),(g,i2)] = mix_pre[h,g] * delta(i1,i2)
    w1 = np.einsum("hg,ab->hagb", m1, eye).reshape(KP, KP).astype(bf16)
    w2 = np.einsum("hg,ab->hagb", m2, eye).reshape(KP, KP).astype(bf16)
    bo_ = np.asarray(bo, np.float32).astype(bf16)

    wsec = np.zeros((128, WSEC_COLS), bf16)
    off = 0

    def chunks(m, width):  # [768, width] -> [128, 6*width]
        return m.reshape(NCC, 128, width).transpose(1, 0, 2).reshape(128, NCC * width)

    wsec[:, off:off + NCC * INNER] = chunks(wq, INNER); off += NCC * INNER
    wsec[:, off:off + NCC * INNER] = chunks(wk, INNER); off += NCC * INNER
    wsec[:, off:off + NCC * INNER] = chunks(wv, INNER); off += NCC * INNER
    wsec[:, off:off + NCC * DIM] = chunks(wo, DIM); off += NCC * DIM
    wsec[0:KP, off:off + KP] = w1; off += KP
    wsec[0:KP, off:off + KP] = w2; off += KP
    wsec[:, off:off + DIM] = np.broadcast_to(bo_, (128, DIM)); off += DIM
    assert off == WSEC_COLS
    return wsec


def _prep_inputs(x, Wq, Wkv, mix_pre, mix_post, Wo, bo):
    bf16 = ml_dtypes.bfloat16
    wsec = _prep_weights(Wq, Wkv, mix_pre, mix_post, Wo, bo)
    x = np.asarray(x, np.float32)
    in_maps = []
    for b in range(B):
        blob = np.empty((128, BLOB_COLS), bf16)
        xt = x[b].T.astype(bf16)  # [768, 1024]
        blob[:, :C_WSH] = xt.reshape(NCC, 128, N).transpose(1, 0, 2).reshape(128, NCC * N)
        blob[:, C_WSH:] = wsec[:, b * WSH_COLS:(b + 1) * WSH_COLS]
        in_maps.append({"blob": blob})
    return in_maps


def _get_nc():
    if "nc" not in _cache:
        _cache["nc"] = _build()
    return _cache["nc"]


def _get_runner():
    """Persistent jitted 8-core runner (jit built once, reused every call)."""
    if "runner" in _cache:
        return _cache["runner"]
    import jax
    from concourse import bass2jax
    from jax.sharding import Mesh, PartitionSpec
    from jax.experimental.shard_map import shard_map

    nc = _get_nc()
    bass2jax.install_neuronx_cc_hook()
    out_aval = jax.core.ShapedArray((N, DIM), ml_dtypes.bfloat16)

    def _bass_body(blob_arg, out_zero):
        outs = bass2jax._bass_exec_p.bind(
            blob_arg, out_zero,
            out_avals=(out_aval,),
            in_names=("blob", "out"),
            out_names=("out",),
            lowering_input_output_aliases=(),
            sim_require_finite=True,
            sim_require_nnan=True,
            nc=nc,
        )
        return tuple(outs)

    devices = jax.devices()[:B]
    mesh = Mesh(np.asarray(devices), ("core",))
    fn = jax.jit(shard_map(_bass_body, mesh=mesh,
                           in_specs=(PartitionSpec("core"),) * 2,
                           out_specs=(PartitionSpec("core"),),
                           check_rep=False),
                 keep_unused=True)
    zeros = np.zeros((B * N, DIM), ml_dtypes.bfloat16)

    def run(blobs):  # blobs: [B*128, BLOB_COLS] bf16
        out = fn(blobs, zeros)[0]
        return np.asarray(out).reshape(B, N, DIM)

    _cache["runner"] = run
    return run


def kernel(x, Wq, Wkv, mix_pre, mix_post, Wo, bo):
    run = _get_runner()
    in_maps = _prep_inputs(x, Wq, Wkv, mix_pre, mix_post, Wo, bo)
    blobs = np.concatenate([m["blob"] for m in in_maps], axis=0)
    return run(blobs).astype(np.float32)
